# revision 14
# baseline (speedup 1.0000x reference)
"""APPNP GNN kernel for 8 TRN2 NeuronCores.

Reference computation (N=100000 nodes, E=1600000 edges, K=5, alpha=0.5):
    h0 = x @ W1 + b1
    deg[d] = |in-edges(d)| + 1 (self loop); dinv = rsqrt(deg)
    5x: h = (1-a) * dinv * S(dinv * h) + a * h0     (S = adjacency sum + self)
    out = relu(h) @ W2 + b2

Device strategy (per core, nodes row-sharded 12500/core padded to 12544):
    track g_t = dinv*h_t.  Per iteration:
      AllGather g (3.2MB/core) -> g_full in local HBM
      msum = scatter_add over in-edges of dma_gather'd g rows (256B/row)
      g_{t+1} = (1-a)*dinv^2*msum + a*g0          (all on DVE)
    msum is pre-initialized with g_t (self loop term).
    Epilogue: u = relu(dinv_h*msum + a*h0); out = u @ W2 + b2 (PE transpose).

Scatter-add safety: the SWDGE ucode assigns descriptor lane
lane(j) = 2*((j%32)//4) + ((j%128)//64) of slot j to DMA engine lane; all
edges of one dst are placed in slots of a single lane so their HBM
read-modify-write adds serialize on one engine.
"""

import math
import numpy as np

# ----------------------------------------------------------------- config

class Cfg:
    def __init__(self, N=100000, E=1600000, F=500, H=64, O=40, K=5, alpha=0.5,
                 cores=8, batch_slots=4096):
        self.N, self.E, self.F, self.H, self.O, self.K = N, E, F, H, O, K
        self.alpha = alpha
        self.cores = cores
        assert N % cores == 0
        self.NP = N // cores                      # real nodes per core
        self.PL = ((self.NP + 127) // 128) * 128  # padded nodes per core
        self.T = self.PL // 128                   # tiles per core
        self.FP = ((F + 127) // 128) * 128        # padded feature dim
        self.KT = self.FP // 128                  # k tiles for lin1
        self.CH = 2 * self.PL                     # gather-chunk rows (int16 safe)
        assert self.CH <= 32600
        self.NCH = cores // 2                     # chunks
        self.GR = cores * self.PL                 # g_full rows
        self.MR = self.PL + 16                    # msum rows (16 lane trash rows)
        self.BS = batch_slots                     # gather/scatter batch slots
        assert self.BS % 128 == 0
        # SWDGE ring carveout is 1024 descriptors; scatter TX uses 2*n/16+1
        assert self.BS // 8 + 1 <= 1024 and self.BS // 16 + 1 <= 1024

FULL = Cfg()

# ----------------------------------------------------------- host preprocess

def host_prep(cfg, x, edge_index, W1, b1, W2, b2):
    """Shard + build layered dst-unique gather/scatter call tables.

    dma_scatter_add's RMW is split across two DMA rings, so a dst row may
    appear at most ONCE per call, and calls that share a dst row need >= 1
    intervening completed call (the slot ping-pong provides it).  Edge k of a
    (dst, chunk) bucket goes to layer k; one call per (chunk, layer),
    split to <= BS slots.
    """
    N, H, F = cfg.N, cfg.H, cfg.F
    src = np.asarray(edge_index[0], dtype=np.int64)
    dst = np.asarray(edge_index[1], dtype=np.int64)

    deg = np.bincount(dst, minlength=N).astype(np.float64) + 1.0
    dinv = (1.0 / np.sqrt(deg)).astype(np.float32)

    src_pad = (src // cfg.NP) * cfg.PL + (src % cfg.NP)
    chunk = src_pad // cfg.CH
    src_loc = (src_pad % cfg.CH).astype(np.int64)
    core_of_dst = dst // cfg.NP
    dst_loc = (dst % cfg.NP).astype(np.int64)

    # layer = occurrence rank within (core, chunk, dst)
    bkey = (core_of_dst * cfg.NCH + chunk) * cfg.PL + dst_loc
    o1 = np.argsort(bkey, kind="stable")
    ks = bkey[o1]
    first = np.r_[True, ks[1:] != ks[:-1]]
    starts = np.where(first, np.arange(len(ks)), 0)
    layer_s = np.arange(len(ks)) - np.maximum.accumulate(starts)
    Lmax = int(layer_s.max()) + 1

    cocs = core_of_dst[o1]
    chs = chunk[o1]
    ckey = (cocs * cfg.NCH + chs) * Lmax + layer_s
    counts = np.bincount(ckey, minlength=cfg.cores * cfg.NCH * Lmax).reshape(
        cfg.cores, cfg.NCH, Lmax)
    P = counts.max(axis=0)                        # [NCH, Lmax]
    P = ((P + 127) // 128) * 128                  # pad to x128 (0 stays 0)

    # call table (chunk, slot_off, nb, first_of_chunk) + per-(c,m) slot offset
    batches = []
    slot_off = np.zeros((cfg.NCH, Lmax), dtype=np.int64)
    off = 0
    for c in range(cfg.NCH):
        fc = True
        for m in range(Lmax):
            if P[c, m] == 0:
                continue
            slot_off[c, m] = off
            rem = int(P[c, m])
            while rem > 0:
                nb = min(cfg.BS, rem)
                batches.append((c, off, nb, fc))
                fc = False
                off += nb
                rem -= nb
    S_total = off

    # second stable sort by (core, chunk, layer); dst order preserved inside
    o2 = np.argsort(ckey, kind="stable")
    src_f = src_loc[o1][o2]
    dst_f = dst_loc[o1][o2]
    ckey_f = ckey[o2]
    bound = np.searchsorted(ckey_f, np.arange(cfg.cores * cfg.NCH * Lmax + 1))

    gidx_all, sidx_all = [], []
    for co in range(cfg.cores):
        gi = np.zeros(S_total, dtype=np.int16)
        si = np.full(S_total, cfg.PL, dtype=np.int16)   # pads -> trash row
        for c in range(cfg.NCH):
            for m in range(Lmax):
                if P[c, m] == 0:
                    continue
                k = (co * cfg.NCH + c) * Lmax + m
                lo, hi = bound[k], bound[k + 1]
                n = hi - lo
                sbase = slot_off[c, m]
                gi[sbase:sbase + n] = src_f[lo:hi].astype(np.int16)
                si[sbase:sbase + n] = dst_f[lo:hi].astype(np.int16)
        gidx_all.append(gi)
        sidx_all.append(si)

    def wrap16(arr):
        w = arr.reshape(-1, 16).T.astype(np.int16)      # [16, S]
        return np.ascontiguousarray(np.tile(w, (8, 1)))  # [128, S]

    def tileize(v):  # [PL] -> [128, T]
        return np.ascontiguousarray(v.reshape(cfg.T, 128).T)

    in_maps = []
    xT = np.zeros((cfg.FP, cfg.PL), dtype=np.float32)
    W1p = np.zeros((cfg.FP, H), dtype=np.float32)
    W1p[:F] = W1.astype(np.float32)
    b1r = np.ascontiguousarray(np.broadcast_to(b1.astype(np.float32), (128, H)))
    b2r = np.ascontiguousarray(np.broadcast_to(b2.astype(np.float32), (128, cfg.O)))
    ident = np.eye(128, dtype=np.float32)

    for c in range(cfg.cores):
        xs = x[c * cfg.NP:(c + 1) * cfg.NP].astype(np.float32)
        xTc = xT.copy()
        xTc[:F, :cfg.NP] = xs.T
        dv = np.zeros(cfg.PL, dtype=np.float32)
        dv[:cfg.NP] = dinv[c * cfg.NP:(c + 1) * cfg.NP]
        rdv = np.zeros(cfg.PL, dtype=np.float32)
        rdv[:cfg.NP] = 1.0 / dv[:cfg.NP]
        in_maps.append({
            "xT": xTc,
            "w1": W1p,
            "b1r": b1r,
            "w2": W2.astype(np.float32),
            "b2r": b2r,
            "ident": ident,
            "dinv": tileize(dv),
            "dinvh": tileize((1.0 - cfg.alpha) * dv),
            "dinv2h": tileize((1.0 - cfg.alpha) * dv * dv),
            "rdinv": tileize(rdv),
            "gidx": wrap16(gidx_all[c]),
            "sidx": wrap16(sidx_all[c]),
        })
    return in_maps, S_total, batches


# ------------------------------------------------------------- graph builder

def build_graph(cfg, S_total, batches, compile_for_hw=True):
    import concourse.bass as bass
    import concourse.bacc as bacc
    import concourse.mybir as mybir
    from concourse.library_config import mlp

    f32 = mybir.dt.float32
    i16 = mybir.dt.int16
    H, O, T, PL = cfg.H, cfg.O, cfg.T, cfg.PL
    NB = len(batches)                     # calls per iteration
    S16 = S_total // 16

    nc = bacc.Bacc("TRN2", target_bir_lowering=False, debug=False,
                   num_devices=cfg.cores, num_swdge_queues=2)

    xT_h = nc.declare_dram_parameter("xT", [cfg.FP, PL], f32, isOutput=False)
    w1_h = nc.declare_dram_parameter("w1", [cfg.FP, H], f32, isOutput=False)
    b1r_h = nc.declare_dram_parameter("b1r", [128, H], f32, isOutput=False)
    w2_h = nc.declare_dram_parameter("w2", [H, O], f32, isOutput=False)
    b2r_h = nc.declare_dram_parameter("b2r", [128, O], f32, isOutput=False)
    id_h = nc.declare_dram_parameter("ident", [128, 128], f32, isOutput=False)
    dinv_h = nc.declare_dram_parameter("dinv", [128, T], f32, isOutput=False)
    dinvh_h = nc.declare_dram_parameter("dinvh", [128, T], f32, isOutput=False)
    dinv2h_h = nc.declare_dram_parameter("dinv2h", [128, T], f32, isOutput=False)
    rdinv_h = nc.declare_dram_parameter("rdinv", [128, T], f32, isOutput=False)
    gidx_h = nc.declare_dram_parameter("gidx", [128, S16], i16, isOutput=False)
    sidx_h = nc.declare_dram_parameter("sidx", [128, S16], i16, isOutput=False)
    out_h = nc.declare_dram_parameter("out", [T, 128, O], f32, isOutput=True)

    cc_in = nc.dram_tensor("cc_in", [PL, H], f32)
    g_full = nc.dram_tensor("g_full", [cfg.GR, H], f32, addr_space="Shared")
    msum = nc.dram_tensor("msum", [cfg.MR, H], f32)

    # lin1 m-groups
    MGW = []
    rem = PL
    while rem > 0:
        w = min(512, rem)
        MGW.append(w)
        rem -= w


    rg = [list(range(cfg.cores))]

    ctxs = []

    def sb(name, shape, dtype):
        cm = nc.sbuf_tensor(name, shape, dtype)
        h = cm.__enter__()
        ctxs.append(cm)
        return h

    def ps(name, shape, dtype):
        cm = nc.psum_tensor(name, shape, dtype)
        h = cm.__enter__()
        ctxs.append(cm)
        return h

    def sem(name):
        cm = nc.semaphore(name)
        h = cm.__enter__()
        ctxs.append(cm)
        return h

    # per-partition f32 per stage slot: gather needs BS*H/128, lin1 needs KT*512
    SLOT_F32 = max(cfg.BS * H // 128, 512 * cfg.KT)
    stage = sb("stage", [128, 2 * SLOT_F32], f32)
    idxg_sb = sb("idxg", [128, S16], i16)
    idxs_sb = sb("idxs", [128, S16], i16)
    g0a_sb = sb("g0a", [128, T, H], f32)
    g_sb = sb("g", [128, T, H], f32)
    msum_sb = sb("msum_sb", [128, T, H], f32)
    w1_sb = sb("w1_sb", [128, cfg.KT, H], f32)
    w2_sb = sb("w2_sb", [H, O], f32)
    b1r_sb = sb("b1r_sb", [128, H], f32)
    b2r_sb = sb("b2r_sb", [128, O], f32)
    id_sb = sb("id_sb", [128, 128], f32)
    dinv_sb = sb("dinv_sb", [128, T], f32)
    dinvh_sb = sb("dinvh_sb", [128, T], f32)
    dinv2h_sb = sb("dinv2h_sb", [128, T], f32)
    rdinv_sb = sb("rdinv_sb", [128, T], f32)
    tmp_sb = sb("tmp_sb", [128, 2, H], f32)
    u_sb = sb("u_sb", [128, 2, H], f32)
    ur_sb = sb("ur_sb", [128, 2, H], f32)
    ah_sb = sb("ah_sb", [128, 2, H], f32)
    lhsT_sb = sb("lhsT_sb", [H, 2, 128], f32)
    OG = 14                                   # out-group tiles
    out_sb = sb("out_sb", [128, 2, OG, O], f32)

    ps_mm = [ps("ps_mm0", [128, H], f32), ps("ps_mm1", [128, H], f32)]
    ps_tr = [ps("ps_tr0", [H, 128], f32), ps("ps_tr1", [H, 128], f32)]
    ps_o = [ps("ps_o0", [128, O], f32), ps("ps_o1", [128, O], f32)]

    s_in = sem("s_in")
    s_x = [sem("s_x0"), sem("s_x1")]
    s_mm = sem("s_mm")
    s_ppf = sem("s_ppf")
    s_ep = sem("s_ep")
    s_gw = sem("s_gw")
    s_cc = sem("s_cc")
    s_gd = [sem("s_gd0"), sem("s_gd1")]
    s_sd = [sem("s_sd0"), sem("s_sd1")]
    s_mr = sem("s_mr")
    s_gu = sem("s_gu")
    s_ah = sem("s_ah")
    s_u = sem("s_u")
    s_ur = sem("s_ur")
    s_tr = sem("s_tr")
    s_trc = sem("s_trc")
    s_mo = sem("s_mo")
    s_ob = sem("s_ob")
    s_ow = [sem("s_ow0"), sem("s_ow1")]

    def stage_slot_g(s, nslots):             # [128, nslots, H] gather/scatter view
        return stage[:, s * SLOT_F32:(s * SLOT_F32 + nslots * H)].rearrange(
            "p (n m) -> p n m", m=H)

    def stage_slot_x(s, kt, w):              # [128, kt, w] lin1 xT view
        return stage[:, s * SLOT_F32:s * SLOT_F32 + kt * 512].rearrange(
            "p (k m) -> p k m", k=kt)[:, :, :w]

    xT3 = xT_h.ap().rearrange("(k p) m -> p k m", p=128)
    w13 = w1_h.ap().rearrange("(k p) m -> p k m", p=128)
    cc3 = cc_in.ap().rearrange("(n p) m -> p n m", p=128)
    ms3 = msum[0:PL, :].rearrange("(n p) m -> p n m", p=128)

    n_in_dmas = 11

    # cumulative tile counts by end of each m-group
    tiles_per_group = [(w + 127) // 128 for w in MGW]
    cum_tiles = np.cumsum([0] + tiles_per_group)

    # ============================== setup block ==============================
    with nc.Block() as blk:
        @blk.sync
        def _(sy):
            sy.dma_start(idxg_sb[:, :], gidx_h[:, :]).then_inc(s_in, 16)
            sy.dma_start(idxs_sb[:, :], sidx_h[:, :]).then_inc(s_in, 16)
            sy.dma_start(w1_sb[:, :, :], w13).then_inc(s_in, 16)
            sy.dma_start(w2_sb[:, :], w2_h[:, :]).then_inc(s_in, 16)
            sy.dma_start(b1r_sb[:, :], b1r_h[:, :]).then_inc(s_in, 16)
            sy.dma_start(b2r_sb[:, :], b2r_h[:, :]).then_inc(s_in, 16)
            sy.dma_start(id_sb[:, :], id_h[:, :]).then_inc(s_in, 16)
            sy.dma_start(dinv_sb[:, :], dinv_h[:, :]).then_inc(s_in, 16)
            sy.dma_start(dinvh_sb[:, :], dinvh_h[:, :]).then_inc(s_in, 16)
            sy.dma_start(dinv2h_sb[:, :], dinv2h_h[:, :]).then_inc(s_in, 16)
            sy.dma_start(rdinv_sb[:, :], rdinv_h[:, :]).then_inc(s_in, 16)
            sy.wait_ge(s_in, 16 * n_in_dmas)
            col = 0
            for mg, w in enumerate(MGW):
                if mg >= 2:
                    # slot reuse: PE must have finished group mg-2
                    sy.wait_ge(s_mm, int(cum_tiles[mg - 1]))
                sy.dma_start(stage_slot_x(mg % 2, cfg.KT, w),
                             xT3[:, :, col:col + w]).then_inc(s_x[mg % 2], 16)
                col += w
            sy.wait_ge(s_ep, T)
            sy.dma_start(cc3, g_sb[:, :, :]).then_inc(s_gw, 16)
            sy.dma_start(ms3, g_sb[:, :, :]).then_inc(s_gw, 16)
            sy.wait_ge(s_gw, 32)

        @blk.tensor
        def _(pe):
            pe.wait_ge(s_in, 16 * n_in_dmas)
            ti = 0
            for mg, w in enumerate(MGW):
                pe.wait_ge(s_x[mg % 2], 16 * (mg // 2 + 1))
                nt = tiles_per_group[mg]
                for m in range(nt):
                    mw = min(128, w - m * 128)
                    if ti >= 2:
                        pe.wait_ge(s_ppf, ti - 1)
                    for k in range(cfg.KT):
                        ins = pe.matmul(
                            ps_mm[ti % 2][:mw, :],
                            stage_slot_x(mg % 2, cfg.KT, w)[:, k, m * 128:m * 128 + mw],
                            w1_sb[:, k, :],
                            start=(k == 0), stop=(k == cfg.KT - 1))
                        if k == cfg.KT - 1:
                            ins.then_inc(s_mm)
                    ti += 1

        @blk.vector
        def _(ve):
            ve.wait_ge(s_in, 16 * n_in_dmas)
            for ti in range(T):
                ve.wait_ge(s_mm, ti + 1)
                if ti >= 2:
                    ve.wait_ge(s_ep, ti - 1)   # Act consumed tmp slot
                ve.tensor_add(tmp_sb[:, ti % 2, :], ps_mm[ti % 2][:, :],
                              b1r_sb[:, :]).then_inc(s_ppf)

        @blk.scalar
        def _(ac):
            ac.wait_ge(s_in, 16 * n_in_dmas)
            for ti in range(T):
                ac.wait_ge(s_ppf, ti + 1)
                ac.activation(g0a_sb[:, ti, :], tmp_sb[:, ti % 2, :],
                              mybir.ActivationFunctionType.Copy,
                              scale=dinvh_sb[:, ti:ti + 1])
                ac.activation(g_sb[:, ti, :], tmp_sb[:, ti % 2, :],
                              mybir.ActivationFunctionType.Copy,
                              scale=dinv_sb[:, ti:ti + 1]).then_inc(s_ep)

        @blk.gpsimd
        def _(gp):
            gp.load_library(mlp)
            gp.wait_ge(s_gw, 32)
            gp.collective_compute(
                "AllGather", mybir.AluOpType.bypass, replica_groups=rg,
                ins=[cc_in.ap().opt()], outs=[g_full.ap().opt()],
            ).then_inc(s_cc)

    # ============================ iteration blocks ===========================
    for t in range(cfg.K):
        last = (t == cfg.K - 1)
        with nc.Block() as blk:
            @blk.gpsimd
            def _(gp, t=t, last=last):
                gp.wait_ge(s_cc, t + 1)
                for gb, (ch, off, nb, fc) in enumerate(batches):
                    G = t * NB + gb
                    if G >= 2:
                        gp.wait_ge(s_sd[G % 2], 16 * (G // 2))
                    gp.dma_gather(
                        stage_slot_g(G % 2, nb // 128),
                        g_full[ch * cfg.CH:(ch + 1) * cfg.CH, :],
                        idxg_sb[:, off // 16:(off + nb) // 16],
                        nb, nb, H, elem_step=H, queue_num=0,
                        single_packet=False,
                    ).then_inc(s_gd[G % 2], 16)
                    if gb >= 1:
                        pch, poff, pnb, pfc = batches[gb - 1]
                        P = G - 1
                        if pfc and pch > 0:
                            # chunk boundary: scatter P-1 may share dst rows
                            gp.wait_ge(s_sd[(P - 1) % 2], 16 * ((P - 1) // 2 + 1))
                        gp.wait_ge(s_gd[P % 2], 16 * (P // 2 + 1))
                        gp.dma_scatter_add(
                            msum.ap(),
                            stage_slot_g(P % 2, pnb // 128),
                            idxs_sb[:, poff // 16:(poff + pnb) // 16],
                            pnb, pnb, H, elem_step=H, queue_num=1,
                            single_packet=False,
                        ).then_inc(s_sd[P % 2], 16)
                # flush last scatter
                lch, loff, lnb, lfc = batches[-1]
                P = t * NB + NB - 1
                gp.wait_ge(s_gd[P % 2], 16 * (P // 2 + 1))
                gp.dma_scatter_add(
                    msum.ap(), stage_slot_g(P % 2, lnb // 128),
                    idxs_sb[:, loff // 16:(loff + lnb) // 16],
                    lnb, lnb, H, elem_step=H, queue_num=1,
                    single_packet=False,
                ).then_inc(s_sd[P % 2], 16)
                if not last:
                    gp.wait_ge(s_gw, 32 * (t + 2))
                    gp.collective_compute(
                        "AllGather", mybir.AluOpType.bypass, replica_groups=rg,
                        ins=[cc_in.ap().opt()], outs=[g_full.ap().opt()],
                    ).then_inc(s_cc)

            @blk.sync
            def _(sy, t=t, last=last):
                tot = NB * (t + 1)
                sy.wait_ge(s_sd[0], 16 * ((tot + 1) // 2))
                sy.wait_ge(s_sd[1], 16 * (tot // 2))
                sy.dma_start(msum_sb[:, :, :], ms3).then_inc(s_mr, 16)
                if not last:
                    sy.wait_ge(s_gu, T * (t + 1))
                    sy.dma_start(cc3, g_sb[:, :, :]).then_inc(s_gw, 16)
                    sy.dma_start(ms3, g_sb[:, :, :]).then_inc(s_gw, 16)
                    sy.wait_ge(s_gw, 32 * (t + 2))
                else:
                    sy.wait_ge(s_mr, 16 * (t + 1))

            if not last:
                @blk.vector
                def _(ve, t=t):
                    ve.wait_ge(s_mr, 16 * (t + 1))
                    for ti in range(T):
                        ve.scalar_tensor_tensor(
                            g_sb[:, ti, :], msum_sb[:, ti, :],
                            dinv2h_sb[:, ti:ti + 1], g0a_sb[:, ti, :],
                            mybir.AluOpType.mult, mybir.AluOpType.add,
                        ).then_inc(s_gu)

    # ============================== epilogue =================================
    with nc.Block() as blk:
        @blk.scalar
        def _(ac):
            for ti in range(T):
                if ti >= 2:
                    ac.wait_ge(s_u, ti - 1)     # DVE consumed ah slot
                ac.activation(ah_sb[:, ti % 2, :], g0a_sb[:, ti, :],
                              mybir.ActivationFunctionType.Copy,
                              scale=rdinv_sb[:, ti:ti + 1]).then_inc(s_ah)
                ac.wait_ge(s_u, ti + 1)
                if ti >= 2:
                    ac.wait_ge(s_tr, ti - 1)    # PE consumed ur slot
                ac.activation(ur_sb[:, ti % 2, :], u_sb[:, ti % 2, :],
                              mybir.ActivationFunctionType.Relu).then_inc(s_ur)

        @blk.vector
        def _(ve):
            ve.wait_ge(s_mr, 16 * cfg.K)
            for ti in range(T):
                og, oslot = ti // OG, (ti // OG) % 2
                if ti % OG == 0 and og >= 2:
                    ve.wait_ge(s_ow[og % 2], 16 * (og // 2))
                ve.wait_ge(s_ah, ti + 1)
                if ti >= 2:
                    ve.wait_ge(s_ur, ti - 1)    # Act consumed u slot
                ve.scalar_tensor_tensor(
                    u_sb[:, ti % 2, :], msum_sb[:, ti, :],
                    dinvh_sb[:, ti:ti + 1], ah_sb[:, ti % 2, :],
                    mybir.AluOpType.mult, mybir.AluOpType.add).then_inc(s_u)
                ve.wait_ge(s_tr, ti + 1)
                ve.tensor_copy(lhsT_sb[:, ti % 2, :],
                               ps_tr[ti % 2][:, :]).then_inc(s_trc)
                ve.wait_ge(s_mo, ti + 1)
                ve.tensor_add(out_sb[:, oslot, ti % OG, :], ps_o[ti % 2][:, :],
                              b2r_sb[:, :]).then_inc(s_ob)

        @blk.tensor
        def _(pe):
            for ti in range(T):
                pe.wait_ge(s_ur, ti + 1)
                if ti >= 2:
                    pe.wait_ge(s_trc, ti - 1)
                pe.transpose(ps_tr[ti % 2][:, :], ur_sb[:, ti % 2, :],
                             id_sb[:, :]).then_inc(s_tr)
                pe.wait_ge(s_trc, ti + 1)
                if ti >= 2:
                    pe.wait_ge(s_ob, ti - 1)
                pe.matmul(ps_o[ti % 2][:, :], lhsT_sb[:, ti % 2, :],
                          w2_sb[:, :], start=True, stop=True).then_inc(s_mo)

        @blk.sync
        def _(sy):
            ngroups = (T + OG - 1) // OG
            for og in range(ngroups):
                t0 = og * OG
                nt = min(OG, T - t0)
                sy.wait_ge(s_ob, t0 + nt)
                dst = out_h[t0:t0 + nt, :, :].rearrange("n p m -> p n m")
                sy.dma_start(dst, out_sb[:, og % 2, 0:nt, :]).then_inc(
                    s_ow[og % 2], 16)
            for par in range(2):
                n_par = (ngroups + 1 - par) // 2
                if n_par:
                    sy.wait_ge(s_ow[par], 16 * n_par)

    if compile_for_hw:
        nc.compile()
    return nc


# ----------------------------------------------------------------- kernel()

_CACHE = {}


def _run(cfg, inputs, trace=False):
    from concourse.bass_utils import run_bass_kernel_spmd

    in_maps, S_total, batches = host_prep(cfg, inputs["x"], inputs["edge_index"],
                                          inputs["W1"], inputs["b1"],
                                          inputs["W2"], inputs["b2"])
    key = (cfg.N, cfg.E, S_total, tuple(b[:3] for b in batches))
    if key not in _CACHE:
        _CACHE[key] = build_graph(cfg, S_total, batches)
    nc = _CACHE[key]
    res = run_bass_kernel_spmd(nc, in_maps, list(range(cfg.cores)), trace=trace)
    outs = []
    for c in range(cfg.cores):
        o = np.asarray(res.results[c]["out"]).reshape(cfg.PL, cfg.O)
        outs.append(o[:cfg.NP])
    return np.concatenate(outs, axis=0), res


def kernel(**inputs):
    out, _ = _run(FULL, inputs)
    return out


# revision 15
# speedup vs baseline: 1.1351x; 1.1351x over previous
"""APPNP GNN kernel for 8 TRN2 NeuronCores.

Reference computation (N=100000 nodes, E=1600000 edges, K=5, alpha=0.5):
    h0 = x @ W1 + b1
    deg[d] = |in-edges(d)| + 1 (self loop); dinv = rsqrt(deg)
    5x: h = (1-a) * dinv * S(dinv * h) + a * h0     (S = adjacency sum + self)
    out = relu(h) @ W2 + b2

Device strategy (per core, nodes row-sharded 12500/core padded to 12544):
    track g_t = dinv*h_t.  Per iteration:
      AllGather g (3.2MB/core) -> g_full in local HBM
      msum = scatter_add over in-edges of dma_gather'd g rows (256B/row)
      g_{t+1} = (1-a)*dinv^2*msum + a*g0          (all on DVE)
    msum is pre-initialized with g_t (self loop term).
    Epilogue: u = relu(dinv_h*msum + a*h0); out = u @ W2 + b2 (PE transpose).

Scatter-add safety: the SWDGE ucode assigns descriptor lane
lane(j) = 2*((j%32)//4) + ((j%128)//64) of slot j to DMA engine lane; all
edges of one dst are placed in slots of a single lane so their HBM
read-modify-write adds serialize on one engine.
"""

import math
import numpy as np

# ----------------------------------------------------------------- config

class Cfg:
    def __init__(self, N=100000, E=1600000, F=500, H=64, O=40, K=5, alpha=0.5,
                 cores=8, batch_slots=4096):
        self.N, self.E, self.F, self.H, self.O, self.K = N, E, F, H, O, K
        self.alpha = alpha
        self.cores = cores
        assert N % cores == 0
        self.NP = N // cores                      # real nodes per core
        self.PL = ((self.NP + 127) // 128) * 128  # padded nodes per core
        self.T = self.PL // 128                   # tiles per core
        self.FP = ((F + 127) // 128) * 128        # padded feature dim
        self.KT = self.FP // 128                  # k tiles for lin1
        self.CH = 2 * self.PL                     # gather-chunk rows (int16 safe)
        assert self.CH <= 32600
        self.NCH = cores // 2                     # chunks
        self.GR = cores * self.PL                 # g_full rows
        self.MR = self.PL + 16                    # msum rows (16 lane trash rows)
        self.DMA_SCRATCH = 32768                  # SWDGE ring carveout bytes
        self.BS = batch_slots                     # gather/scatter batch slots
        assert self.BS % 128 == 0
        # SWDGE ring carveout is 1024 descriptors; scatter TX uses 2*n/16+1
        assert self.BS // 8 + 1 <= 1024 and self.BS // 16 + 1 <= 1024

FULL = Cfg()

# ----------------------------------------------------------- host preprocess

def host_prep(cfg, x, edge_index, W1, b1, W2, b2):
    """Shard + build layered dst-unique gather/scatter call tables.

    dma_scatter_add's RMW is split across two DMA rings, so a dst row may
    appear at most ONCE per call, and calls that share a dst row need >= 1
    intervening completed call (the slot ping-pong provides it).  Edge k of a
    (dst, chunk) bucket goes to layer k; one call per (chunk, layer),
    split to <= BS slots.
    """
    N, H, F = cfg.N, cfg.H, cfg.F
    src = np.asarray(edge_index[0], dtype=np.int64)
    dst = np.asarray(edge_index[1], dtype=np.int64)

    deg = np.bincount(dst, minlength=N).astype(np.float64) + 1.0
    dinv = (1.0 / np.sqrt(deg)).astype(np.float32)

    src_pad = (src // cfg.NP) * cfg.PL + (src % cfg.NP)
    chunk = src_pad // cfg.CH
    src_loc = (src_pad % cfg.CH).astype(np.int64)
    core_of_dst = dst // cfg.NP
    dst_loc = (dst % cfg.NP).astype(np.int64)

    # layer = occurrence rank within (core, chunk, dst)
    bkey = (core_of_dst * cfg.NCH + chunk) * cfg.PL + dst_loc
    o1 = np.argsort(bkey, kind="stable")
    ks = bkey[o1]
    first = np.r_[True, ks[1:] != ks[:-1]]
    starts = np.where(first, np.arange(len(ks)), 0)
    layer_s = np.arange(len(ks)) - np.maximum.accumulate(starts)
    Lmax = int(layer_s.max()) + 1

    cocs = core_of_dst[o1]
    chs = chunk[o1]
    ckey = (cocs * cfg.NCH + chs) * Lmax + layer_s
    counts = np.bincount(ckey, minlength=cfg.cores * cfg.NCH * Lmax).reshape(
        cfg.cores, cfg.NCH, Lmax)
    P = counts.max(axis=0)                        # [NCH, Lmax]
    P = ((P + 127) // 128) * 128                  # pad to x128 (0 stays 0)

    # call table (chunk, slot_off, nb, first_of_chunk) + per-(c,m) slot offset
    batches = []
    slot_off = np.zeros((cfg.NCH, Lmax), dtype=np.int64)
    off = 0
    for c in range(cfg.NCH):
        fc = True
        for m in range(Lmax):
            if P[c, m] == 0:
                continue
            slot_off[c, m] = off
            rem = int(P[c, m])
            while rem > 0:
                nb = min(cfg.BS, rem)
                batches.append((c, off, nb, fc))
                fc = False
                off += nb
                rem -= nb
    S_total = off

    # second stable sort by (core, chunk, layer); dst order preserved inside
    o2 = np.argsort(ckey, kind="stable")
    src_f = src_loc[o1][o2]
    dst_f = dst_loc[o1][o2]
    ckey_f = ckey[o2]
    bound = np.searchsorted(ckey_f, np.arange(cfg.cores * cfg.NCH * Lmax + 1))

    gidx_all, sidx_all = [], []
    for co in range(cfg.cores):
        gi = np.zeros(S_total, dtype=np.int16)
        si = np.full(S_total, cfg.PL, dtype=np.int16)   # pads -> trash row
        for c in range(cfg.NCH):
            for m in range(Lmax):
                if P[c, m] == 0:
                    continue
                k = (co * cfg.NCH + c) * Lmax + m
                lo, hi = bound[k], bound[k + 1]
                n = hi - lo
                sbase = slot_off[c, m]
                gi[sbase:sbase + n] = src_f[lo:hi].astype(np.int16)
                si[sbase:sbase + n] = dst_f[lo:hi].astype(np.int16)
        gidx_all.append(gi)
        sidx_all.append(si)

    def wrap16(arr):
        w = arr.reshape(-1, 16).T.astype(np.int16)      # [16, S]
        return np.ascontiguousarray(np.tile(w, (8, 1)))  # [128, S]

    def tileize(v):  # [PL] -> [128, T]
        return np.ascontiguousarray(v.reshape(cfg.T, 128).T)

    in_maps = []
    xT = np.zeros((cfg.FP, cfg.PL), dtype=np.float32)
    W1p = np.zeros((cfg.FP, H), dtype=np.float32)
    W1p[:F] = W1.astype(np.float32)
    b1r = np.ascontiguousarray(np.broadcast_to(b1.astype(np.float32), (128, H)))
    b2r = np.ascontiguousarray(np.broadcast_to(b2.astype(np.float32), (128, cfg.O)))
    ident = np.eye(128, dtype=np.float32)

    for c in range(cfg.cores):
        xs = x[c * cfg.NP:(c + 1) * cfg.NP].astype(np.float32)
        xTc = xT.copy()
        xTc[:F, :cfg.NP] = xs.T
        dv = np.zeros(cfg.PL, dtype=np.float32)
        dv[:cfg.NP] = dinv[c * cfg.NP:(c + 1) * cfg.NP]
        rdv = np.zeros(cfg.PL, dtype=np.float32)
        rdv[:cfg.NP] = 1.0 / dv[:cfg.NP]
        in_maps.append({
            "xT": xTc,
            "w1": W1p,
            "b1r": b1r,
            "w2": W2.astype(np.float32),
            "b2r": b2r,
            "ident": ident,
            "dinv": tileize(dv),
            "dinvh": tileize((1.0 - cfg.alpha) * dv),
            "dinv2h": tileize((1.0 - cfg.alpha) * dv * dv),
            "rdinv": tileize(rdv),
            "gidx": wrap16(gidx_all[c]),
            "sidx": wrap16(sidx_all[c]),
        })
    return in_maps, S_total, batches


# ------------------------------------------------------------- graph builder

def build_graph(cfg, S_total, batches, compile_for_hw=True):
    import concourse.bass as bass
    import concourse.bacc as bacc
    import concourse.mybir as mybir
    from concourse.library_config import mlp

    f32 = mybir.dt.float32
    i16 = mybir.dt.int16
    H, O, T, PL = cfg.H, cfg.O, cfg.T, cfg.PL
    NB = len(batches)                     # calls per iteration
    S16 = S_total // 16

    nc = bacc.Bacc("TRN2", target_bir_lowering=False, debug=False,
                   num_devices=cfg.cores, num_swdge_queues=2,
                   dynamic_dma_scratch_size=cfg.DMA_SCRATCH)

    xT_h = nc.declare_dram_parameter("xT", [cfg.FP, PL], f32, isOutput=False)
    w1_h = nc.declare_dram_parameter("w1", [cfg.FP, H], f32, isOutput=False)
    b1r_h = nc.declare_dram_parameter("b1r", [128, H], f32, isOutput=False)
    w2_h = nc.declare_dram_parameter("w2", [H, O], f32, isOutput=False)
    b2r_h = nc.declare_dram_parameter("b2r", [128, O], f32, isOutput=False)
    id_h = nc.declare_dram_parameter("ident", [128, 128], f32, isOutput=False)
    dinv_h = nc.declare_dram_parameter("dinv", [128, T], f32, isOutput=False)
    dinvh_h = nc.declare_dram_parameter("dinvh", [128, T], f32, isOutput=False)
    dinv2h_h = nc.declare_dram_parameter("dinv2h", [128, T], f32, isOutput=False)
    rdinv_h = nc.declare_dram_parameter("rdinv", [128, T], f32, isOutput=False)
    gidx_h = nc.declare_dram_parameter("gidx", [128, S16], i16, isOutput=False)
    sidx_h = nc.declare_dram_parameter("sidx", [128, S16], i16, isOutput=False)
    out_h = nc.declare_dram_parameter("out", [T, 128, O], f32, isOutput=True)

    cc_in = nc.dram_tensor("cc_in", [PL, H], f32)
    g_full = nc.dram_tensor("g_full", [cfg.GR, H], f32, addr_space="Shared")
    msum = nc.dram_tensor("msum", [cfg.MR, H], f32)

    # lin1 m-groups
    MGW = []
    rem = PL
    while rem > 0:
        w = min(512, rem)
        MGW.append(w)
        rem -= w


    rg = [list(range(cfg.cores))]

    ctxs = []

    def sb(name, shape, dtype):
        cm = nc.sbuf_tensor(name, shape, dtype)
        h = cm.__enter__()
        ctxs.append(cm)
        return h

    def ps(name, shape, dtype):
        cm = nc.psum_tensor(name, shape, dtype)
        h = cm.__enter__()
        ctxs.append(cm)
        return h

    def sem(name):
        cm = nc.semaphore(name)
        h = cm.__enter__()
        ctxs.append(cm)
        return h

    # per-partition f32 per stage slot: gather needs BS*H/128, lin1 needs KT*512
    SLOT_F32 = max(cfg.BS * H // 128, 512 * cfg.KT)
    stage = sb("stage", [128, 2 * SLOT_F32], f32)
    idxg_sb = sb("idxg", [128, S16], i16)
    idxs_sb = sb("idxs", [128, S16], i16)
    g0a_sb = sb("g0a", [128, T, H], f32)
    g_sb = sb("g", [128, T, H], f32)
    msum_sb = sb("msum_sb", [128, T, H], f32)
    w1_sb = sb("w1_sb", [128, cfg.KT, H], f32)
    w2_sb = sb("w2_sb", [H, O], f32)
    b1r_sb = sb("b1r_sb", [128, H], f32)
    b2r_sb = sb("b2r_sb", [128, O], f32)
    id_sb = sb("id_sb", [128, 128], f32)
    dinv_sb = sb("dinv_sb", [128, T], f32)
    dinvh_sb = sb("dinvh_sb", [128, T], f32)
    dinv2h_sb = sb("dinv2h_sb", [128, T], f32)
    rdinv_sb = sb("rdinv_sb", [128, T], f32)
    tmp_sb = sb("tmp_sb", [128, 2, H], f32)
    u_sb = sb("u_sb", [128, 2, H], f32)
    ur_sb = sb("ur_sb", [128, 2, H], f32)
    ah_sb = sb("ah_sb", [128, 2, H], f32)
    lhsT_sb = sb("lhsT_sb", [H, 2, 128], f32)
    OG = 14                                   # out-group tiles
    out_sb = sb("out_sb", [128, 2, OG, O], f32)

    ps_mm = [ps("ps_mm0", [128, H], f32), ps("ps_mm1", [128, H], f32)]
    ps_tr = [ps("ps_tr0", [H, 128], f32), ps("ps_tr1", [H, 128], f32)]
    ps_o = [ps("ps_o0", [128, O], f32), ps("ps_o1", [128, O], f32)]

    s_in = sem("s_in")
    s_x = [sem("s_x0"), sem("s_x1")]
    s_mm = sem("s_mm")
    s_ppf = sem("s_ppf")
    s_ep = sem("s_ep")
    s_gw = sem("s_gw")
    s_cc = sem("s_cc")
    s_gd = [sem("s_gd0"), sem("s_gd1")]
    s_sd = [sem("s_sd0"), sem("s_sd1")]
    s_mr = sem("s_mr")
    s_gu = sem("s_gu")
    s_ah = sem("s_ah")
    s_u = sem("s_u")
    s_ur = sem("s_ur")
    s_tr = sem("s_tr")
    s_trc = sem("s_trc")
    s_mo = sem("s_mo")
    s_ob = sem("s_ob")
    s_ow = [sem("s_ow0"), sem("s_ow1")]

    def stage_slot_g(s, nslots):             # [128, nslots, H] gather/scatter view
        return stage[:, s * SLOT_F32:(s * SLOT_F32 + nslots * H)].rearrange(
            "p (n m) -> p n m", m=H)

    def stage_slot_x(s, kt, w):              # [128, kt, w] lin1 xT view
        return stage[:, s * SLOT_F32:s * SLOT_F32 + kt * 512].rearrange(
            "p (k m) -> p k m", k=kt)[:, :, :w]

    xT3 = xT_h.ap().rearrange("(k p) m -> p k m", p=128)
    w13 = w1_h.ap().rearrange("(k p) m -> p k m", p=128)
    cc3 = cc_in.ap().rearrange("(n p) m -> p n m", p=128)
    ms3 = msum[0:PL, :].rearrange("(n p) m -> p n m", p=128)

    n_in_dmas = 11

    # cumulative tile counts by end of each m-group
    tiles_per_group = [(w + 127) // 128 for w in MGW]
    cum_tiles = np.cumsum([0] + tiles_per_group)

    # ============================== setup block ==============================
    with nc.Block() as blk:
        @blk.sync
        def _(sy):
            sy.dma_start(idxg_sb[:, :], gidx_h[:, :]).then_inc(s_in, 16)
            sy.dma_start(idxs_sb[:, :], sidx_h[:, :]).then_inc(s_in, 16)
            sy.dma_start(w1_sb[:, :, :], w13).then_inc(s_in, 16)
            sy.dma_start(w2_sb[:, :], w2_h[:, :]).then_inc(s_in, 16)
            sy.dma_start(b1r_sb[:, :], b1r_h[:, :]).then_inc(s_in, 16)
            sy.dma_start(b2r_sb[:, :], b2r_h[:, :]).then_inc(s_in, 16)
            sy.dma_start(id_sb[:, :], id_h[:, :]).then_inc(s_in, 16)
            sy.dma_start(dinv_sb[:, :], dinv_h[:, :]).then_inc(s_in, 16)
            sy.dma_start(dinvh_sb[:, :], dinvh_h[:, :]).then_inc(s_in, 16)
            sy.dma_start(dinv2h_sb[:, :], dinv2h_h[:, :]).then_inc(s_in, 16)
            sy.dma_start(rdinv_sb[:, :], rdinv_h[:, :]).then_inc(s_in, 16)
            sy.wait_ge(s_in, 16 * n_in_dmas)
            col = 0
            for mg, w in enumerate(MGW):
                if mg >= 2:
                    # slot reuse: PE must have finished group mg-2
                    sy.wait_ge(s_mm, int(cum_tiles[mg - 1]))
                sy.dma_start(stage_slot_x(mg % 2, cfg.KT, w),
                             xT3[:, :, col:col + w]).then_inc(s_x[mg % 2], 16)
                col += w
            sy.wait_ge(s_ep, T)
            sy.dma_start(cc3, g_sb[:, :, :]).then_inc(s_gw, 16)
            sy.dma_start(ms3, g_sb[:, :, :]).then_inc(s_gw, 16)
            sy.wait_ge(s_gw, 32)

        @blk.tensor
        def _(pe):
            pe.wait_ge(s_in, 16 * n_in_dmas)
            ti = 0
            for mg, w in enumerate(MGW):
                pe.wait_ge(s_x[mg % 2], 16 * (mg // 2 + 1))
                nt = tiles_per_group[mg]
                for m in range(nt):
                    mw = min(128, w - m * 128)
                    if ti >= 2:
                        pe.wait_ge(s_ppf, ti - 1)
                    for k in range(cfg.KT):
                        ins = pe.matmul(
                            ps_mm[ti % 2][:mw, :],
                            stage_slot_x(mg % 2, cfg.KT, w)[:, k, m * 128:m * 128 + mw],
                            w1_sb[:, k, :],
                            start=(k == 0), stop=(k == cfg.KT - 1))
                        if k == cfg.KT - 1:
                            ins.then_inc(s_mm)
                    ti += 1

        @blk.vector
        def _(ve):
            ve.wait_ge(s_in, 16 * n_in_dmas)
            for ti in range(T):
                ve.wait_ge(s_mm, ti + 1)
                if ti >= 2:
                    ve.wait_ge(s_ep, ti - 1)   # Act consumed tmp slot
                ve.tensor_add(tmp_sb[:, ti % 2, :], ps_mm[ti % 2][:, :],
                              b1r_sb[:, :]).then_inc(s_ppf)

        @blk.scalar
        def _(ac):
            ac.wait_ge(s_in, 16 * n_in_dmas)
            for ti in range(T):
                ac.wait_ge(s_ppf, ti + 1)
                ac.activation(g0a_sb[:, ti, :], tmp_sb[:, ti % 2, :],
                              mybir.ActivationFunctionType.Copy,
                              scale=dinvh_sb[:, ti:ti + 1])
                ac.activation(g_sb[:, ti, :], tmp_sb[:, ti % 2, :],
                              mybir.ActivationFunctionType.Copy,
                              scale=dinv_sb[:, ti:ti + 1]).then_inc(s_ep)

        @blk.gpsimd
        def _(gp):
            gp.load_library(mlp)
            gp.wait_ge(s_gw, 32)
            gp.collective_compute(
                "AllGather", mybir.AluOpType.bypass, replica_groups=rg,
                ins=[cc_in.ap().opt()], outs=[g_full.ap().opt()],
            ).then_inc(s_cc)

    # ============================ iteration blocks ===========================
    for t in range(cfg.K):
        last = (t == cfg.K - 1)
        with nc.Block() as blk:
            @blk.gpsimd
            def _(gp, t=t, last=last):
                gp.wait_ge(s_cc, t + 1)
                for gb, (ch, off, nb, fc) in enumerate(batches):
                    G = t * NB + gb
                    if G >= 2:
                        gp.wait_ge(s_sd[G % 2], 16 * (G // 2))
                    gp.dma_gather(
                        stage_slot_g(G % 2, nb // 128),
                        g_full[ch * cfg.CH:(ch + 1) * cfg.CH, :],
                        idxg_sb[:, off // 16:(off + nb) // 16],
                        nb, nb, H, elem_step=H, queue_num=0,
                        single_packet=False,
                    ).then_inc(s_gd[G % 2], 16)
                    if gb >= 1:
                        pch, poff, pnb, pfc = batches[gb - 1]
                        P = G - 1
                        if pfc and pch > 0:
                            # chunk boundary: scatter P-1 may share dst rows
                            gp.wait_ge(s_sd[(P - 1) % 2], 16 * ((P - 1) // 2 + 1))
                        gp.wait_ge(s_gd[P % 2], 16 * (P // 2 + 1))
                        gp.dma_scatter_add(
                            msum.ap(),
                            stage_slot_g(P % 2, pnb // 128),
                            idxs_sb[:, poff // 16:(poff + pnb) // 16],
                            pnb, pnb, H, elem_step=H, queue_num=1,
                            single_packet=False,
                        ).then_inc(s_sd[P % 2], 16)
                # flush last scatter
                lch, loff, lnb, lfc = batches[-1]
                P = t * NB + NB - 1
                gp.wait_ge(s_gd[P % 2], 16 * (P // 2 + 1))
                gp.dma_scatter_add(
                    msum.ap(), stage_slot_g(P % 2, lnb // 128),
                    idxs_sb[:, loff // 16:(loff + lnb) // 16],
                    lnb, lnb, H, elem_step=H, queue_num=1,
                    single_packet=False,
                ).then_inc(s_sd[P % 2], 16)
                if not last:
                    gp.wait_ge(s_gw, 32 * (t + 2))
                    gp.collective_compute(
                        "AllGather", mybir.AluOpType.bypass, replica_groups=rg,
                        ins=[cc_in.ap().opt()], outs=[g_full.ap().opt()],
                    ).then_inc(s_cc)

            @blk.sync
            def _(sy, t=t, last=last):
                tot = NB * (t + 1)
                sy.wait_ge(s_sd[0], 16 * ((tot + 1) // 2))
                sy.wait_ge(s_sd[1], 16 * (tot // 2))
                sy.dma_start(msum_sb[:, :, :], ms3).then_inc(s_mr, 16)
                if not last:
                    sy.wait_ge(s_gu, T * (t + 1))
                    sy.dma_start(cc3, g_sb[:, :, :]).then_inc(s_gw, 16)
                    sy.dma_start(ms3, g_sb[:, :, :]).then_inc(s_gw, 16)
                    sy.wait_ge(s_gw, 32 * (t + 2))
                else:
                    sy.wait_ge(s_mr, 16 * (t + 1))

            if not last:
                @blk.vector
                def _(ve, t=t):
                    ve.wait_ge(s_mr, 16 * (t + 1))
                    for ti in range(T):
                        ve.scalar_tensor_tensor(
                            g_sb[:, ti, :], msum_sb[:, ti, :],
                            dinv2h_sb[:, ti:ti + 1], g0a_sb[:, ti, :],
                            mybir.AluOpType.mult, mybir.AluOpType.add,
                        ).then_inc(s_gu)

    # ============================== epilogue =================================
    with nc.Block() as blk:
        @blk.scalar
        def _(ac):
            for ti in range(T):
                if ti >= 2:
                    ac.wait_ge(s_u, ti - 1)     # DVE consumed ah slot
                ac.activation(ah_sb[:, ti % 2, :], g0a_sb[:, ti, :],
                              mybir.ActivationFunctionType.Copy,
                              scale=rdinv_sb[:, ti:ti + 1]).then_inc(s_ah)
                ac.wait_ge(s_u, ti + 1)
                if ti >= 2:
                    ac.wait_ge(s_tr, ti - 1)    # PE consumed ur slot
                ac.activation(ur_sb[:, ti % 2, :], u_sb[:, ti % 2, :],
                              mybir.ActivationFunctionType.Relu).then_inc(s_ur)

        @blk.vector
        def _(ve):
            ve.wait_ge(s_mr, 16 * cfg.K)
            for ti in range(T):
                og, oslot = ti // OG, (ti // OG) % 2
                if ti % OG == 0 and og >= 2:
                    ve.wait_ge(s_ow[og % 2], 16 * (og // 2))
                ve.wait_ge(s_ah, ti + 1)
                if ti >= 2:
                    ve.wait_ge(s_ur, ti - 1)    # Act consumed u slot
                ve.scalar_tensor_tensor(
                    u_sb[:, ti % 2, :], msum_sb[:, ti, :],
                    dinvh_sb[:, ti:ti + 1], ah_sb[:, ti % 2, :],
                    mybir.AluOpType.mult, mybir.AluOpType.add).then_inc(s_u)
                ve.wait_ge(s_tr, ti + 1)
                ve.tensor_copy(lhsT_sb[:, ti % 2, :],
                               ps_tr[ti % 2][:, :]).then_inc(s_trc)
                ve.wait_ge(s_mo, ti + 1)
                ve.tensor_add(out_sb[:, oslot, ti % OG, :], ps_o[ti % 2][:, :],
                              b2r_sb[:, :]).then_inc(s_ob)

        @blk.tensor
        def _(pe):
            for ti in range(T):
                pe.wait_ge(s_ur, ti + 1)
                if ti >= 2:
                    pe.wait_ge(s_trc, ti - 1)
                pe.transpose(ps_tr[ti % 2][:, :], ur_sb[:, ti % 2, :],
                             id_sb[:, :]).then_inc(s_tr)
                pe.wait_ge(s_trc, ti + 1)
                if ti >= 2:
                    pe.wait_ge(s_ob, ti - 1)
                pe.matmul(ps_o[ti % 2][:, :], lhsT_sb[:, ti % 2, :],
                          w2_sb[:, :], start=True, stop=True).then_inc(s_mo)

        @blk.sync
        def _(sy):
            ngroups = (T + OG - 1) // OG
            for og in range(ngroups):
                t0 = og * OG
                nt = min(OG, T - t0)
                sy.wait_ge(s_ob, t0 + nt)
                dst = out_h[t0:t0 + nt, :, :].rearrange("n p m -> p n m")
                sy.dma_start(dst, out_sb[:, og % 2, 0:nt, :]).then_inc(
                    s_ow[og % 2], 16)
            for par in range(2):
                n_par = (ngroups + 1 - par) // 2
                if n_par:
                    sy.wait_ge(s_ow[par], 16 * n_par)

    print(f"SBUF used: {(nc.sbuf_base + (nc.SBUF_PARTITION_SIZE_BYTES - nc.sbuf_top)) / 1024:.0f} KB/part "
          f"(base {nc.sbuf_base//1024}KB top-res {(nc.SBUF_PARTITION_SIZE_BYTES - nc.sbuf_top)//1024}KB of {nc.SBUF_PARTITION_SIZE_BYTES//1024}KB)")
    if compile_for_hw:
        nc.compile()
    return nc


# ----------------------------------------------------------------- kernel()

_CACHE = {}


def _run(cfg, inputs, trace=False):
    from concourse.bass_utils import run_bass_kernel_spmd

    in_maps, S_total, batches = host_prep(cfg, inputs["x"], inputs["edge_index"],
                                          inputs["W1"], inputs["b1"],
                                          inputs["W2"], inputs["b2"])
    key = (cfg.N, cfg.E, S_total, tuple(b[:3] for b in batches))
    if key not in _CACHE:
        _CACHE[key] = build_graph(cfg, S_total, batches)
    nc = _CACHE[key]
    res = run_bass_kernel_spmd(nc, in_maps, list(range(cfg.cores)), trace=trace)
    outs = []
    for c in range(cfg.cores):
        o = np.asarray(res.results[c]["out"]).reshape(cfg.PL, cfg.O)
        outs.append(o[:cfg.NP])
    return np.concatenate(outs, axis=0), res


def kernel(**inputs):
    out, _ = _run(FULL, inputs)
    return out
